# revision 17
# baseline (speedup 1.0000x reference)
"""BVGAE GNN message-passing kernel for 8 TRN2 NeuronCores.

Pipeline (4 SPMD launches; the host relays the small cross-core
activations between launches; each launch is row-sharded over 8 cores):

  L1  x1    : X1 = norm * (h @ W0.T)               (1024 rows/core, bf16)
  L2  spmm1 : S1 = A @ X1 via dense 128x128 one-hot block matmuls;
              h0s = norm * relu(norm * S1)         (dest rows sharded)
  L3  spmm2 : S2 = A @ h0s (same machinery); heads:
              x = norm * (S2 @ (W1.T @ w_ab)) + b;  elu(x)+1.5
  L4  pairs : alpha_p/beta_p row panels out[i,j] = a[i] + a[j] (bf16)

SPMM strategy: the whole 8192x256 bf16 activation table (4.2 MB) stays
resident in SBUF, partition-major ([p, src_tile, hid], p = row % 128).
The adjacency is host-packed into dense uint8 one-hot blocks
oh[dt, p, st, d] = #edges (st*128+p) -> (core*8*128... dt*128+d), one
1 MB contiguous DMA per dest tile.  On device each block is converted
u8 -> bf16 on DVE and scatter-added on the PE:

    S[dt][d, h] += sum_p oh[p, st, d] * tbl[p, st, h]

64 accumulating matmuls per dest tile, no indexed DMA anywhere (the
baseline's per-edge dma_gather was SWDGE descriptor-bound at ~1 ms per
layer).

Math identity for layer 2: h1 @ w = norm * ((A @ h0s) @ (W1.T @ w)), so
the second SPMM needs only the 256-wide h0s table and two projected
weight vectors.

The `reps` builder argument repeats the launch body back-to-back inside
one NEFF; the test harness slope-fits per-launch HW time with it
(wall-clock through the PJRT proxy cannot resolve microseconds).
"""
import os
import numpy as np

import concourse.bass as bass
import concourse.bacc as bacc
import concourse.mybir as mybir
import concourse.tile as tile
from concourse.bass_utils import run_bass_kernel_spmd

F32 = mybir.dt.float32
BF16 = mybir.dt.bfloat16
U8 = mybir.dt.uint8
FP8 = mybir.dt.float8e4
NP_BF16 = mybir.dt.np(BF16)
NP_FP8 = mybir.dt.np(FP8)
# which spmm layers run the table+one-hot in fp8e4m3 with DoubleRow
# (double-pumped 256-deep contraction): "" none, "2" L2 only, "23" both
FP8_LAYERS = os.environ.get("BVGAE_FP8", "23")
AOT = mybir.AluOpType
AFT = mybir.ActivationFunctionType
MS = bass.MemorySpace

N = 8192
F_IN = 512
HID = 256
CORES = 8
RPC = N // CORES          # rows per core
TPC = RPC // 128          # 128-row dest tiles per core
NT = N // 128             # 128-row source tiles (64)
KB = F_IN // 128

LAST_LAUNCHES = []        # (name, builder, in_maps) stashed when BVGAE_KEEP=1


def _run(nc, in_maps, name="", builder=None):
    if os.environ.get("BVGAE_KEEP") == "1":
        LAST_LAUNCHES.append((name, builder, in_maps))
    res = run_bass_kernel_spmd(nc, in_maps, core_ids=list(range(CORES)))
    return res.results


def _norm_tiles(nc, pool, deg_dram):
    """deg [128, TPC] -> norm = 1/sqrt(deg) in SBUF [128, TPC]."""
    deg_sb = pool.tile([128, TPC], F32)
    nc.sync.dma_start(deg_sb[:], deg_dram[:])
    sq = pool.tile([128, TPC], F32)
    nc.scalar.activation(sq[:], deg_sb[:], AFT.Sqrt)
    norm_sb = pool.tile([128, TPC], F32)
    nc.vector.reciprocal(norm_sb[:], sq[:])
    return norm_sb


def _build_l1(reps=1):
    nc = bacc.Bacc("TRN2", target_bir_lowering=False, debug=False,
                   num_devices=CORES)
    ht = nc.dram_tensor("ht", [KB, 128, RPC], BF16, kind="ExternalInput")
    w0t = nc.dram_tensor("w0t", [KB, 128, HID], BF16, kind="ExternalInput")
    deg = nc.dram_tensor("degc", [128, TPC], F32, kind="ExternalInput")
    x1c = nc.dram_tensor("x1c", [RPC, HID], BF16, kind="ExternalOutput")

    with tile.TileContext(nc) as tc:
        with (
            tc.tile_pool(name="pool", bufs=2) as pool,
            tc.tile_pool(name="io", bufs=2) as io,
            tc.tile_pool(name="psum", bufs=2, space=MS.PSUM) as pps,
        ):
            for _rep in range(reps):
                ht_sb = pool.tile([128, KB, RPC], BF16, tag="ht")
                w0_sb = pool.tile([128, KB, HID], BF16, tag="w0")
                for k in range(KB):
                    nc.sync.dma_start(ht_sb[:, k, :], ht[k])
                    nc.sync.dma_start(w0_sb[:, k, :], w0t[k])
                norm_sb = _norm_tiles(nc, pool, deg)

                for t in range(TPC):
                    ps = pps.tile([128, HID], F32, tag="ps")
                    for k in range(KB):
                        nc.tensor.matmul(
                            ps[:], ht_sb[:, k, bass.ts(t, 128)],
                            w0_sb[:, k, :],
                            start=(k == 0), stop=(k == KB - 1),
                        )
                    xt = io.tile([128, HID], BF16, tag="xt")
                    nc.scalar.activation(xt[:], ps[:], AFT.Copy,
                                         scale=norm_sb[:, t:t + 1])
                    nc.sync.dma_start(x1c[bass.ts(t, 128), :], xt[:])
    nc.compile()
    return nc


def _build_spmm(layer, fp8, reps=1):
    """Shared builder for L2 (layer=1) and L3 (layer=2)."""
    nc = bacc.Bacc("TRN2", target_bir_lowering=False, debug=False,
                   num_devices=CORES)
    TDT = FP8 if fp8 else BF16
    tblp = nc.dram_tensor("tblp", [128, NT, HID], TDT, kind="ExternalInput")
    ohc = nc.dram_tensor("ohc", [TPC, 128, NT, 128], FP8 if fp8 else U8,
                         kind="ExternalInput")
    deg = nc.dram_tensor("degc", [128, TPC], F32, kind="ExternalInput")
    if layer == 1:
        h0sc = nc.dram_tensor("h0sc", [RPC, HID], BF16, kind="ExternalOutput")
    else:
        # host-projected head rows (W1.T @ w_{alpha,beta}) replicated
        # across partitions, and the matching biases
        headb = nc.dram_tensor("headb", [128, 2, HID], F32,
                               kind="ExternalInput")
        babb = nc.dram_tensor("babb", [128, 2], F32, kind="ExternalInput")
        abc = nc.dram_tensor("abc", [128, 2, TPC], F32, kind="ExternalOutput")

    with tile.TileContext(nc) as tc:
        with (
            tc.tile_pool(name="pool", bufs=1) as pool,
            tc.tile_pool(name="oh", bufs=2) as ohp,
            tc.tile_pool(name="work", bufs=2) as work,
            tc.tile_pool(name="psum", bufs=2, space=MS.PSUM) as pps,
        ):
            for _rep in range(reps):
                tbl_sb = pool.tile([128, NT, HID], TDT, tag="tbl")
                nc.sync.dma_start(tbl_sb[:], tblp[:])
                norm_sb = _norm_tiles(nc, pool, deg)

                if layer == 2:
                    head_b = pool.tile([128, 2, HID], F32, tag="headb")
                    nc.scalar.dma_start(head_b[:], headb[:])
                    bab_b = pool.tile([128, 2], F32, tag="babb")
                    nc.scalar.dma_start(bab_b[:], babb[:])
                    ab_all = pool.tile([128, 2, TPC], F32, tag="aball")

                for t in range(TPC):
                    ps = pps.tile([128, HID], F32, tag="ps")
                    if fp8:
                        oh_sb = ohp.tile([128, NT, 128], FP8, tag="oh")
                        nc.scalar.dma_start(oh_sb[:], ohc[t])
                        for q in range(NT // 2):
                            nc.tensor.matmul(
                                ps[:], oh_sb[:, 2 * q:2 * q + 2, :],
                                tbl_sb[:, 2 * q:2 * q + 2, :],
                                start=(q == 0), stop=(q == NT // 2 - 1),
                                perf_mode=mybir.MatmulPerfMode.DoubleRow)
                    else:
                        oh_sb = ohp.tile([128, NT, 128], U8, tag="oh")
                        nc.sync.dma_start(oh_sb[:], ohc[t])
                        oh_bf = ohp.tile([128, NT, 128], BF16, tag="ohbf")
                        nc.vector.tensor_copy(oh_bf[:], oh_sb[:])
                        for st in range(NT):
                            nc.tensor.matmul(ps[:], oh_bf[:, st, :],
                                             tbl_sb[:, st, :],
                                             start=(st == 0),
                                             stop=(st == NT - 1))

                    nt = norm_sb[:, t:t + 1]
                    if layer == 1:
                        rt = work.tile([128, HID], F32, tag="rt")
                        nc.scalar.activation(rt[:], ps[:], AFT.Relu, scale=nt)
                        h0t = work.tile([128, HID], BF16, tag="h0t")
                        nc.vector.tensor_scalar(h0t[:], rt[:], nt, None,
                                                op0=AOT.mult)
                        nc.sync.dma_start(h0sc[bass.ts(t, 128), :], h0t[:])
                    else:
                        s2s = work.tile([128, HID], F32, tag="s2s")
                        nc.vector.tensor_copy(s2s[:], ps[:])
                        junk = work.tile([128, HID], F32, tag="junk")
                        sab = work.tile([128, 2], F32, tag="sab")
                        # NB: fused tensor_tensor_reduce crashes HW here;
                        # use separate mult + reduce
                        for hd in range(2):
                            nc.vector.tensor_tensor(junk[:], s2s[:],
                                                    head_b[:, hd, :],
                                                    op=AOT.mult)
                            nc.vector.tensor_reduce(sab[:, hd:hd + 1],
                                                    junk[:],
                                                    mybir.AxisListType.X,
                                                    AOT.add)
                        x = work.tile([128, 2], F32, tag="x")
                        for hd in range(2):
                            nc.vector.tensor_scalar(
                                x[:, hd:hd + 1], sab[:, hd:hd + 1], nt,
                                bab_b[:, hd:hd + 1],
                                op0=AOT.mult, op1=AOT.add)
                        # elu(x) + 1.5 = exp(min(x,0)) + max(x,0) + 0.5
                        mn = work.tile([128, 2], F32, tag="mn")
                        nc.vector.tensor_scalar(mn[:], x[:], 0.0, None,
                                                op0=AOT.min)
                        ex = work.tile([128, 2], F32, tag="ex")
                        nc.scalar.activation(ex[:], mn[:], AFT.Exp)
                        mx = work.tile([128, 2], F32, tag="mx")
                        nc.vector.tensor_scalar(mx[:], x[:], 0.0, 0.5,
                                                op0=AOT.max, op1=AOT.add)
                        nc.vector.tensor_tensor(ab_all[:, :, t], ex[:],
                                                mx[:], op=AOT.add)
                if layer == 2:
                    nc.sync.dma_start(abc[:], ab_all[:])
    nc.compile()
    return nc


NBLK = 33                 # col blocks written per row tile (mod-64 distance)
WTRI = NBLK * 128         # 4224 cols per row tile
EXT = (TPC - 1) * 128 + WTRI  # per-core (rolled) broadcast width, 5120


def _build_l4(tri, reps=1):
    """Pairwise panels.  tri=True writes, for global row tile t, only the
    col blocks t..t+32 (mod 64) into a dense [128, WTRI] stripe (every
    unordered block pair lands on exactly one writer; host mirrors the
    rest via transposes).  The alpha/beta row broadcasts come in
    pre-replicated from the host."""
    nc = bacc.Bacc("TRN2", target_bir_lowering=False, debug=False,
                   num_devices=CORES)
    W = EXT if tri else N
    OW = WTRI if tri else N
    abc = nc.dram_tensor("abc", [128, W], BF16, kind="ExternalInput")
    bbc = nc.dram_tensor("bbc", [128, W], BF16, kind="ExternalInput")
    act = nc.dram_tensor("act", [128, TPC], F32, kind="ExternalInput")
    bct = nc.dram_tensor("bct", [128, TPC], F32, kind="ExternalInput")
    arows = nc.dram_tensor("arows", [RPC, OW], BF16, kind="ExternalOutput")
    brows = nc.dram_tensor("brows", [RPC, OW], BF16, kind="ExternalOutput")

    with tile.TileContext(nc) as tc:
        with (
            tc.tile_pool(name="pool", bufs=1) as pool,
            tc.tile_pool(name="out", bufs=2) as outp,
        ):
            for _rep in range(reps):
                act_sb = pool.tile([128, TPC], F32, tag="act")
                nc.sync.dma_start(act_sb[:], act[:])
                bct_sb = pool.tile([128, TPC], F32, tag="bct")
                nc.scalar.dma_start(bct_sb[:], bct[:])
                bca = pool.tile([128, W], BF16, tag="bca")
                nc.sync.dma_start(bca[:], abc[:])
                bcb = pool.tile([128, W], BF16, tag="bcb")
                nc.scalar.dma_start(bcb[:], bbc[:])

                for t in range(TPC):
                    # per-core col offset handled host-side: for tri the
                    # bcast input is pre-rolled so local tile lt starts at
                    # col lt*128
                    c0 = t * 128 if tri else 0
                    # both adds on DVE; the two output streams drain on the
                    # two independent HWDGE rings (SP + ACT)
                    oa = outp.tile([128, OW], BF16, tag="oa")
                    nc.vector.tensor_scalar(oa[:], bca[:, c0:c0 + OW],
                                            act_sb[:, t:t + 1], None,
                                            op0=AOT.add)
                    nc.sync.dma_start(arows[bass.ts(t, 128), :], oa[:])
                    ob = outp.tile([128, OW], BF16, tag="ob")
                    nc.vector.tensor_scalar(ob[:], bcb[:, c0:c0 + OW],
                                            bct_sb[:, t:t + 1], None,
                                            op0=AOT.add)
                    nc.scalar.dma_start(brows[bass.ts(t, 128), :], ob[:])
    nc.compile()
    return nc


def _prep_onehot(row, col):
    """Dense per-core one-hot blocks oh[dt, p, st, d] (uint8 edge counts)
    for dest node (core*TPC + dt)*128 + d, source node st*128 + p."""
    g = row >> 7                      # dest 128-row tile id, 0..63
    core = g >> 3
    dt = g & (TPC - 1)
    st = col >> 7
    p = col & 127
    d = row & 127
    flat = ((dt.astype(np.int64) * 128 + p) * NT + st) * 128 + d
    ohs = []
    for c in range(CORES):
        f = flat[core == c]
        cnt = np.bincount(f, minlength=TPC * 128 * NT * 128)
        ohs.append(cnt.astype(np.uint8).reshape(TPC, 128, NT, 128))
    return ohs


def _table_pmajor(x):
    """(N, HID) -> partition-major [128, NT, HID] with [p, st, :] =
    x[st*128 + p]."""
    return np.ascontiguousarray(
        x.reshape(NT, 128, HID).transpose(1, 0, 2))


_cache = {}


def _get(name, builder, *args):
    key = (name,) + args
    if key not in _cache:
        _cache[key] = builder(*args)
    return _cache[key]


def kernel(row, col, h, W0, W1, w_alpha, b_alpha, w_beta, b_beta):
    LAST_LAUNCHES.clear()
    row = np.asarray(row)
    col = np.asarray(col)
    h = np.asarray(h, np.float32)
    W0 = np.asarray(W0, np.float32)
    W1 = np.asarray(W1, np.float32)

    deg = np.bincount(row, minlength=N).astype(np.float32)
    degc = [np.ascontiguousarray(deg[c * RPC:(c + 1) * RPC]
                                 .reshape(TPC, 128).T) for c in range(CORES)]
    ohs = _prep_onehot(row, col)

    # ---- L1: X1 = norm * (h @ W0.T) ----
    hT = np.ascontiguousarray(h.T).astype(NP_BF16)     # (512, 8192)
    w0t = np.ascontiguousarray(
        W0.T.reshape(KB, 128, HID)).astype(NP_BF16)
    nc1 = _get("l1", _build_l1)
    in1 = [{
        "ht": np.ascontiguousarray(
            hT[:, c * RPC:(c + 1) * RPC].reshape(KB, 128, RPC)),
        "w0t": w0t,
        "degc": degc[c],
    } for c in range(CORES)]
    r1 = _run(nc1, in1, "l1", _build_l1)
    x1 = np.concatenate([r1[c]["x1c"] for c in range(CORES)])

    # ---- L2: h0s = norm * relu(norm * (A @ X1)) ----
    fp8_2 = "2" in FP8_LAYERS
    nc2 = _get("spmm", _build_spmm, 1, fp8_2)
    tbl1 = _table_pmajor(x1)
    oh2 = [o.astype(NP_FP8) for o in ohs] if fp8_2 else ohs
    in2 = [{"tblp": tbl1.astype(NP_FP8) if fp8_2 else tbl1,
            "ohc": oh2[c], "degc": degc[c]}
           for c in range(CORES)]
    r2 = _run(nc2, in2, "l2", lambda reps=1: _build_spmm(1, fp8_2, reps))
    h0s = np.concatenate([r2[c]["h0sc"] for c in range(CORES)])

    # ---- L3: alpha/beta = elu(norm*((A @ h0s) @ (W1.T @ w)) + b) + 1.5 ----
    fp8_3 = "3" in FP8_LAYERS
    nc3 = _get("spmm", _build_spmm, 2, fp8_3)
    tbl2 = _table_pmajor(h0s)
    oh3 = [o.astype(NP_FP8) for o in ohs] if fp8_3 else ohs
    wa1 = W1.T @ np.asarray(w_alpha, np.float32)          # (HID,)
    wb1 = W1.T @ np.asarray(w_beta, np.float32)
    headb_in = np.ascontiguousarray(
        np.broadcast_to(np.stack([wa1, wb1]), (128, 2, HID)))
    babb_in = np.ascontiguousarray(np.broadcast_to(
        np.array([np.asarray(b_alpha).reshape(-1)[0],
                  np.asarray(b_beta).reshape(-1)[0]], np.float32), (128, 2)))
    in3 = [{"tblp": tbl2.astype(NP_FP8) if fp8_3 else tbl2,
            "ohc": oh3[c], "degc": degc[c],
            "headb": headb_in, "babb": babb_in}
           for c in range(CORES)]
    r3 = _run(nc3, in3, "l3", lambda reps=1: _build_spmm(2, fp8_3, reps))
    # abc[p, hd, t] -> value for node c*RPC + t*128 + p
    alpha = np.concatenate(
        [r3[c]["abc"][:, 0, :].T.reshape(-1) for c in range(CORES)])
    beta = np.concatenate(
        [r3[c]["abc"][:, 1, :].T.reshape(-1) for c in range(CORES)])

    # ---- L4: pairwise broadcast-sum panels ----
    tri = os.environ.get("BVGAE_TRI", "1") == "1"
    nc4 = _get("l4", _build_l4, tri)
    acts = [np.ascontiguousarray(
        alpha[c * RPC:(c + 1) * RPC].reshape(TPC, 128).T)
        for c in range(CORES)]
    bcts = [np.ascontiguousarray(
        beta[c * RPC:(c + 1) * RPC].reshape(TPC, 128).T)
        for c in range(CORES)]
    if tri:
        a2 = np.concatenate([alpha, alpha]).astype(NP_BF16)
        b2 = np.concatenate([beta, beta]).astype(NP_BF16)
        in4 = [{
            "abc": np.ascontiguousarray(np.broadcast_to(
                a2[c * RPC:c * RPC + EXT], (128, EXT))),
            "bbc": np.ascontiguousarray(np.broadcast_to(
                b2[c * RPC:c * RPC + EXT], (128, EXT))),
            "act": acts[c], "bct": bcts[c],
        } for c in range(CORES)]
    else:
        abc_in = np.ascontiguousarray(
            np.broadcast_to(alpha.astype(NP_BF16), (128, N)))
        bbc_in = np.ascontiguousarray(
            np.broadcast_to(beta.astype(NP_BF16), (128, N)))
        in4 = [{"abc": abc_in, "bbc": bbc_in, "act": acts[c],
                "bct": bcts[c]} for c in range(CORES)]
    r4 = _run(nc4, in4, "l4", lambda reps=1: _build_l4(tri, reps))
    arows = np.concatenate([r4[c]["arows"] for c in range(CORES)])
    brows = np.concatenate([r4[c]["brows"] for c in range(CORES)])
    if not tri:
        return arows.astype(np.float32), brows.astype(np.float32)

    outs = []
    for rows in (arows, brows):
        P = np.empty((N, N), np.float32)
        for t in range(NT):
            rs = slice(t * 128, (t + 1) * 128)
            c0 = t * 128
            c1 = c0 + WTRI
            blk = rows[rs].astype(np.float32)
            if c1 <= N:
                P[rs, c0:c1] = blk
            else:
                P[rs, c0:] = blk[:, :N - c0]
                P[rs, :c1 - N] = blk[:, N - c0:]
        # mirror the unwritten blocks from their transposed twins
        for t in range(NT):
            rs = slice(t * 128, (t + 1) * 128)
            for db in range(NBLK, NT):
                s = (t + db) % NT
                cs = slice(s * 128, (s + 1) * 128)
                P[rs, cs] = P[cs, rs].T
        outs.append(P)
    return outs[0], outs[1]


# revision 23
# speedup vs baseline: 1.4572x; 1.4572x over previous
"""BVGAE GNN message-passing kernel for 8 TRN2 NeuronCores.

Pipeline (4 SPMD launches; the host relays the small cross-core
activations between launches; each launch is row-sharded over 8 cores):

  L1  x1    : X1 = norm * (h @ W0.T)               (1024 rows/core, bf16)
  L2  spmm1 : S1 = A @ X1 via dense 128x128 one-hot block matmuls;
              h0s = norm * relu(norm * S1)         (dest rows sharded)
  L3  spmm2 : S2 = A @ h0s (same machinery); heads:
              x = norm * (S2 @ (W1.T @ w_ab)) + b;  elu(x)+1.5
  L4  pairs : alpha_p/beta_p row panels out[i,j] = a[i] + a[j] (bf16)

SPMM strategy: the whole 8192x256 bf16 activation table (4.2 MB) stays
resident in SBUF, partition-major ([p, src_tile, hid], p = row % 128).
The adjacency is host-packed into dense uint8 one-hot blocks
oh[dt, p, st, d] = #edges (st*128+p) -> (core*8*128... dt*128+d), one
1 MB contiguous DMA per dest tile.  On device each block is converted
u8 -> bf16 on DVE and scatter-added on the PE:

    S[dt][d, h] += sum_p oh[p, st, d] * tbl[p, st, h]

64 accumulating matmuls per dest tile, no indexed DMA anywhere (the
baseline's per-edge dma_gather was SWDGE descriptor-bound at ~1 ms per
layer).

Math identity for layer 2: h1 @ w = norm * ((A @ h0s) @ (W1.T @ w)), so
the second SPMM needs only the 256-wide h0s table and two projected
weight vectors.

The `reps` builder argument repeats the launch body back-to-back inside
one NEFF; the test harness slope-fits per-launch HW time with it
(wall-clock through the PJRT proxy cannot resolve microseconds).
"""
import os
import numpy as np

import concourse.bass as bass
import concourse.bacc as bacc
import concourse.mybir as mybir
import concourse.tile as tile
from concourse.bass_utils import run_bass_kernel_spmd

F32 = mybir.dt.float32
BF16 = mybir.dt.bfloat16
U8 = mybir.dt.uint8
FP8 = mybir.dt.float8e4
NP_BF16 = mybir.dt.np(BF16)
NP_FP8 = mybir.dt.np(FP8)
# which spmm layers run the table+one-hot in fp8e4m3 with DoubleRow
# (double-pumped 256-deep contraction): "" none, "2" L2 only, "23" both
FP8_LAYERS = os.environ.get("BVGAE_FP8", "23")
AOT = mybir.AluOpType
AFT = mybir.ActivationFunctionType
MS = bass.MemorySpace

N = 8192
F_IN = 512
HID = 256
CORES = 8
RPC = N // CORES          # rows per core
TPC = RPC // 128          # 128-row dest tiles per core
NT = N // 128             # 128-row source tiles (64)
KB = F_IN // 128

LAST_LAUNCHES = []        # (name, builder, in_maps) stashed when BVGAE_KEEP=1


def _run(nc, in_maps, name="", builder=None):
    if os.environ.get("BVGAE_KEEP") == "1":
        LAST_LAUNCHES.append((name, builder, in_maps))
    res = run_bass_kernel_spmd(nc, in_maps, core_ids=list(range(CORES)))
    return res.results


def _norm_tiles(nc, pool, deg_dram):
    """deg [128, TPC] -> norm = 1/sqrt(deg) in SBUF [128, TPC]."""
    deg_sb = pool.tile([128, TPC], F32)
    nc.sync.dma_start(deg_sb[:], deg_dram[:])
    sq = pool.tile([128, TPC], F32)
    nc.scalar.activation(sq[:], deg_sb[:], AFT.Sqrt)
    norm_sb = pool.tile([128, TPC], F32)
    nc.vector.reciprocal(norm_sb[:], sq[:])
    return norm_sb


def _build_l1(reps=1):
    nc = bacc.Bacc("TRN2", target_bir_lowering=False, debug=False,
                   num_devices=CORES)
    ht = nc.dram_tensor("ht", [KB, 128, RPC], BF16, kind="ExternalInput")
    w0t = nc.dram_tensor("w0t", [KB, 128, HID], BF16, kind="ExternalInput")
    deg = nc.dram_tensor("degc", [128, TPC], F32, kind="ExternalInput")
    x1c = nc.dram_tensor("x1c", [RPC, HID], BF16, kind="ExternalOutput")

    with tile.TileContext(nc) as tc:
        with (
            tc.tile_pool(name="pool", bufs=2) as pool,
            tc.tile_pool(name="io", bufs=2) as io,
            tc.tile_pool(name="psum", bufs=2, space=MS.PSUM) as pps,
        ):
            for _rep in range(reps):
                ht_sb = pool.tile([128, KB, RPC], BF16, tag="ht")
                w0_sb = pool.tile([128, KB, HID], BF16, tag="w0")
                for k in range(KB):
                    nc.sync.dma_start(ht_sb[:, k, :], ht[k])
                    nc.sync.dma_start(w0_sb[:, k, :], w0t[k])
                norm_sb = _norm_tiles(nc, pool, deg)

                for t in range(TPC):
                    ps = pps.tile([128, HID], F32, tag="ps")
                    for k in range(KB):
                        nc.tensor.matmul(
                            ps[:], ht_sb[:, k, bass.ts(t, 128)],
                            w0_sb[:, k, :],
                            start=(k == 0), stop=(k == KB - 1),
                        )
                    xt = io.tile([128, HID], BF16, tag="xt")
                    nc.scalar.activation(xt[:], ps[:], AFT.Copy,
                                         scale=norm_sb[:, t:t + 1])
                    nc.sync.dma_start(x1c[bass.ts(t, 128), :], xt[:])
    nc.compile()
    return nc


def _build_spmm(layer, fp8, reps=1):
    """Shared builder for L2 (layer=1) and L3 (layer=2)."""
    nc = bacc.Bacc("TRN2", target_bir_lowering=False, debug=False,
                   num_devices=CORES)
    TDT = FP8 if fp8 else BF16
    tblp = nc.dram_tensor("tblp", [128, NT, HID], TDT, kind="ExternalInput")
    ohc = nc.dram_tensor("ohc", [TPC, 128, NT, 128], FP8 if fp8 else U8,
                         kind="ExternalInput")
    deg = nc.dram_tensor("degc", [128, TPC], F32, kind="ExternalInput")
    if layer == 1:
        h0sc = nc.dram_tensor("h0sc", [RPC, HID], BF16, kind="ExternalOutput")
    else:
        # host-projected head rows (W1.T @ w_{alpha,beta}) replicated
        # across partitions, and the matching biases
        headb = nc.dram_tensor("headb", [128, 2, HID], F32,
                               kind="ExternalInput")
        babb = nc.dram_tensor("babb", [128, 2], F32, kind="ExternalInput")
        abc = nc.dram_tensor("abc", [128, 2, TPC], F32, kind="ExternalOutput")

    with tile.TileContext(nc) as tc:
        with (
            tc.tile_pool(name="pool", bufs=1) as pool,
            tc.tile_pool(name="oh", bufs=2) as ohp,
            tc.tile_pool(name="work", bufs=2) as work,
            tc.tile_pool(name="psum", bufs=2, space=MS.PSUM) as pps,
        ):
            for _rep in range(reps):
                tbl_sb = pool.tile([128, NT, HID], TDT, tag="tbl")
                nc.sync.dma_start(tbl_sb[:], tblp[:])
                if layer == 1:
                    # degc carries 1/deg for the fused relu epilogue
                    invd_sb = pool.tile([128, TPC], F32, tag="invd")
                    nc.sync.dma_start(invd_sb[:], deg[:])
                else:
                    norm_sb = _norm_tiles(nc, pool, deg)

                if layer == 2:
                    head_b = pool.tile([128, 2, HID], F32, tag="headb")
                    nc.scalar.dma_start(head_b[:], headb[:])
                    bab_b = pool.tile([128, 2], F32, tag="babb")
                    nc.scalar.dma_start(bab_b[:], babb[:])
                    ab_all = pool.tile([128, 2, TPC], F32, tag="aball")

                for t in range(TPC):
                    ps = pps.tile([128, HID], F32, tag="ps")
                    if fp8:
                        oh_sb = ohp.tile([128, NT, 128], FP8, tag="oh")
                        nc.sync.dma_start(oh_sb[:], ohc[t])
                        for q in range(NT // 2):
                            nc.tensor.matmul(
                                ps[:], oh_sb[:, 2 * q:2 * q + 2, :],
                                tbl_sb[:, 2 * q:2 * q + 2, :],
                                start=(q == 0), stop=(q == NT // 2 - 1),
                                perf_mode=mybir.MatmulPerfMode.DoubleRow)
                    else:
                        oh_sb = ohp.tile([128, NT, 128], U8, tag="oh")
                        nc.sync.dma_start(oh_sb[:], ohc[t])
                        oh_bf = ohp.tile([128, NT, 128], BF16, tag="ohbf")
                        nc.vector.tensor_copy(oh_bf[:], oh_sb[:])
                        for st in range(NT):
                            nc.tensor.matmul(ps[:], oh_bf[:, st, :],
                                             tbl_sb[:, st, :],
                                             start=(st == 0),
                                             stop=(st == NT - 1))

                    if layer == 1:
                        # h0s = norm*relu(norm*S) = relu(S)/deg (norm > 0)
                        h0t = work.tile([128, HID], BF16, tag="h0t")
                        nc.scalar.activation(h0t[:], ps[:], AFT.Relu,
                                             scale=invd_sb[:, t:t + 1])
                        nc.sync.dma_start(h0sc[bass.ts(t, 128), :], h0t[:])
                    else:
                        nt = norm_sb[:, t:t + 1]
                        s2s = work.tile([128, HID], F32, tag="s2s")
                        nc.vector.tensor_copy(s2s[:], ps[:])
                        junk = work.tile([128, HID], F32, tag="junk")
                        sab = work.tile([128, 2], F32, tag="sab")
                        # NB: fused tensor_tensor_reduce crashes HW here;
                        # use separate mult + reduce
                        for hd in range(2):
                            nc.vector.tensor_tensor(junk[:], s2s[:],
                                                    head_b[:, hd, :],
                                                    op=AOT.mult)
                            nc.vector.tensor_reduce(sab[:, hd:hd + 1],
                                                    junk[:],
                                                    mybir.AxisListType.X,
                                                    AOT.add)
                        x = work.tile([128, 2], F32, tag="x")
                        for hd in range(2):
                            nc.vector.tensor_scalar(
                                x[:, hd:hd + 1], sab[:, hd:hd + 1], nt,
                                bab_b[:, hd:hd + 1],
                                op0=AOT.mult, op1=AOT.add)
                        # elu(x) + 1.5 = exp(min(x,0)) + max(x,0) + 0.5
                        mn = work.tile([128, 2], F32, tag="mn")
                        nc.vector.tensor_scalar(mn[:], x[:], 0.0, None,
                                                op0=AOT.min)
                        ex = work.tile([128, 2], F32, tag="ex")
                        nc.scalar.activation(ex[:], mn[:], AFT.Exp)
                        mx = work.tile([128, 2], F32, tag="mx")
                        nc.vector.tensor_scalar(mx[:], x[:], 0.0, 0.5,
                                                op0=AOT.max, op1=AOT.add)
                        nc.vector.tensor_tensor(ab_all[:, :, t], ex[:],
                                                mx[:], op=AOT.add)
                if layer == 2:
                    nc.sync.dma_start(abc[:], ab_all[:])
    nc.compile()
    return nc


NBLK = 33                 # col blocks written per row tile (mod-64 distance)
WTRI = NBLK * 128         # 4224 cols per row tile
EXT = (TPC - 1) * 128 + WTRI  # per-core (rolled) broadcast width, 5120


def _build_l4(tri, reps=1):
    """Pairwise panels.  tri=True writes, for global row tile t, only the
    col blocks t..t+32 (mod 64) into a dense [128, WTRI] stripe (every
    unordered block pair lands on exactly one writer; host mirrors the
    rest via transposes).  The alpha/beta row broadcasts come in
    pre-replicated from the host."""
    nc = bacc.Bacc("TRN2", target_bir_lowering=False, debug=False,
                   num_devices=CORES)
    W = EXT if tri else N
    OW = WTRI if tri else N
    abc = nc.dram_tensor("abc", [128, W], BF16, kind="ExternalInput")
    bbc = nc.dram_tensor("bbc", [128, W], BF16, kind="ExternalInput")
    act = nc.dram_tensor("act", [128, TPC], F32, kind="ExternalInput")
    bct = nc.dram_tensor("bct", [128, TPC], F32, kind="ExternalInput")
    arows = nc.dram_tensor("arows", [RPC, OW], BF16, kind="ExternalOutput")
    brows = nc.dram_tensor("brows", [RPC, OW], BF16, kind="ExternalOutput")

    with tile.TileContext(nc) as tc:
        with (
            tc.tile_pool(name="pool", bufs=1) as pool,
            tc.tile_pool(name="out", bufs=2) as outp,
        ):
            for _rep in range(reps):
                act_sb = pool.tile([128, TPC], F32, tag="act")
                nc.sync.dma_start(act_sb[:], act[:])
                bct_sb = pool.tile([128, TPC], F32, tag="bct")
                nc.scalar.dma_start(bct_sb[:], bct[:])
                bca = pool.tile([128, W], BF16, tag="bca")
                nc.sync.dma_start(bca[:], abc[:])
                bcb = pool.tile([128, W], BF16, tag="bcb")
                nc.scalar.dma_start(bcb[:], bbc[:])

                for t in range(TPC):
                    # per-core col offset handled host-side: for tri the
                    # bcast input is pre-rolled so local tile lt starts at
                    # col lt*128
                    c0 = t * 128 if tri else 0
                    # both adds on DVE; the two output streams drain on the
                    # two independent HWDGE rings (SP + ACT)
                    oa = outp.tile([128, OW], BF16, tag="oa")
                    nc.vector.tensor_scalar(oa[:], bca[:, c0:c0 + OW],
                                            act_sb[:, t:t + 1], None,
                                            op0=AOT.add)
                    nc.sync.dma_start(arows[bass.ts(t, 128), :], oa[:])
                    ob = outp.tile([128, OW], BF16, tag="ob")
                    nc.vector.tensor_scalar(ob[:], bcb[:, c0:c0 + OW],
                                            bct_sb[:, t:t + 1], None,
                                            op0=AOT.add)
                    nc.scalar.dma_start(brows[bass.ts(t, 128), :], ob[:])
    nc.compile()
    return nc


def _prep_onehot(row, col):
    """Dense per-core one-hot blocks oh[dt, p, st, d] (uint8 edge counts)
    for dest node (core*TPC + dt)*128 + d, source node st*128 + p."""
    g = row >> 7                      # dest 128-row tile id, 0..63
    core = g >> 3
    dt = g & (TPC - 1)
    st = col >> 7
    p = col & 127
    d = row & 127
    flat = ((dt.astype(np.int64) * 128 + p) * NT + st) * 128 + d
    ohs = []
    for c in range(CORES):
        f = flat[core == c]
        cnt = np.bincount(f, minlength=TPC * 128 * NT * 128)
        ohs.append(cnt.astype(np.uint8).reshape(TPC, 128, NT, 128))
    return ohs


def _table_pmajor(x):
    """(N, HID) -> partition-major [128, NT, HID] with [p, st, :] =
    x[st*128 + p]."""
    return np.ascontiguousarray(
        x.reshape(NT, 128, HID).transpose(1, 0, 2))


_cache = {}


def _get(name, builder, *args):
    key = (name,) + args
    if key not in _cache:
        _cache[key] = builder(*args)
    return _cache[key]


def kernel(row, col, h, W0, W1, w_alpha, b_alpha, w_beta, b_beta):
    LAST_LAUNCHES.clear()
    row = np.asarray(row)
    col = np.asarray(col)
    h = np.asarray(h, np.float32)
    W0 = np.asarray(W0, np.float32)
    W1 = np.asarray(W1, np.float32)

    deg = np.bincount(row, minlength=N).astype(np.float32)
    degc = [np.ascontiguousarray(deg[c * RPC:(c + 1) * RPC]
                                 .reshape(TPC, 128).T) for c in range(CORES)]
    ohs = _prep_onehot(row, col)

    # ---- L1: X1 = norm * (h @ W0.T) ----
    hT = np.ascontiguousarray(h.T).astype(NP_BF16)     # (512, 8192)
    w0t = np.ascontiguousarray(
        W0.T.reshape(KB, 128, HID)).astype(NP_BF16)
    nc1 = _get("l1", _build_l1)
    in1 = [{
        "ht": np.ascontiguousarray(
            hT[:, c * RPC:(c + 1) * RPC].reshape(KB, 128, RPC)),
        "w0t": w0t,
        "degc": degc[c],
    } for c in range(CORES)]
    r1 = _run(nc1, in1, "l1", _build_l1)
    x1 = np.concatenate([r1[c]["x1c"] for c in range(CORES)])

    # ---- L2: h0s = norm * relu(norm * (A @ X1)) ----
    fp8_2 = "2" in FP8_LAYERS
    nc2 = _get("spmm", _build_spmm, 1, fp8_2)
    tbl1 = _table_pmajor(x1)
    oh2 = [o.astype(NP_FP8) for o in ohs] if fp8_2 else ohs
    in2 = [{"tblp": tbl1.astype(NP_FP8) if fp8_2 else tbl1,
            "ohc": oh2[c], "degc": 1.0 / degc[c]}
           for c in range(CORES)]
    r2 = _run(nc2, in2, "l2", lambda reps=1: _build_spmm(1, fp8_2, reps))
    h0s = np.concatenate([r2[c]["h0sc"] for c in range(CORES)])

    # ---- L3: alpha/beta = elu(norm*((A @ h0s) @ (W1.T @ w)) + b) + 1.5 ----
    fp8_3 = "3" in FP8_LAYERS
    nc3 = _get("spmm", _build_spmm, 2, fp8_3)
    tbl2 = _table_pmajor(h0s)
    oh3 = [o.astype(NP_FP8) for o in ohs] if fp8_3 else ohs
    wa1 = W1.T @ np.asarray(w_alpha, np.float32)          # (HID,)
    wb1 = W1.T @ np.asarray(w_beta, np.float32)
    headb_in = np.ascontiguousarray(
        np.broadcast_to(np.stack([wa1, wb1]), (128, 2, HID)))
    babb_in = np.ascontiguousarray(np.broadcast_to(
        np.array([np.asarray(b_alpha).reshape(-1)[0],
                  np.asarray(b_beta).reshape(-1)[0]], np.float32), (128, 2)))
    in3 = [{"tblp": tbl2.astype(NP_FP8) if fp8_3 else tbl2,
            "ohc": oh3[c], "degc": degc[c],
            "headb": headb_in, "babb": babb_in}
           for c in range(CORES)]
    r3 = _run(nc3, in3, "l3", lambda reps=1: _build_spmm(2, fp8_3, reps))
    # abc[p, hd, t] -> value for node c*RPC + t*128 + p
    alpha = np.concatenate(
        [r3[c]["abc"][:, 0, :].T.reshape(-1) for c in range(CORES)])
    beta = np.concatenate(
        [r3[c]["abc"][:, 1, :].T.reshape(-1) for c in range(CORES)])

    # ---- L4: pairwise broadcast-sum panels ----
    tri = os.environ.get("BVGAE_TRI", "1") == "1"
    nc4 = _get("l4", _build_l4, tri)
    acts = [np.ascontiguousarray(
        alpha[c * RPC:(c + 1) * RPC].reshape(TPC, 128).T)
        for c in range(CORES)]
    bcts = [np.ascontiguousarray(
        beta[c * RPC:(c + 1) * RPC].reshape(TPC, 128).T)
        for c in range(CORES)]
    if tri:
        a2 = np.concatenate([alpha, alpha]).astype(NP_BF16)
        b2 = np.concatenate([beta, beta]).astype(NP_BF16)
        in4 = [{
            "abc": np.ascontiguousarray(np.broadcast_to(
                a2[c * RPC:c * RPC + EXT], (128, EXT))),
            "bbc": np.ascontiguousarray(np.broadcast_to(
                b2[c * RPC:c * RPC + EXT], (128, EXT))),
            "act": acts[c], "bct": bcts[c],
        } for c in range(CORES)]
    else:
        abc_in = np.ascontiguousarray(
            np.broadcast_to(alpha.astype(NP_BF16), (128, N)))
        bbc_in = np.ascontiguousarray(
            np.broadcast_to(beta.astype(NP_BF16), (128, N)))
        in4 = [{"abc": abc_in, "bbc": bbc_in, "act": acts[c],
                "bct": bcts[c]} for c in range(CORES)]
    r4 = _run(nc4, in4, "l4", lambda reps=1: _build_l4(tri, reps))
    arows = np.concatenate([r4[c]["arows"] for c in range(CORES)])
    brows = np.concatenate([r4[c]["brows"] for c in range(CORES)])
    if not tri:
        return arows.astype(np.float32), brows.astype(np.float32)

    outs = []
    for rows in (arows, brows):
        P = np.empty((N, N), np.float32)
        for t in range(NT):
            rs = slice(t * 128, (t + 1) * 128)
            c0 = t * 128
            c1 = c0 + WTRI
            blk = rows[rs].astype(np.float32)
            if c1 <= N:
                P[rs, c0:c1] = blk
            else:
                P[rs, c0:] = blk[:, :N - c0]
                P[rs, :c1 - N] = blk[:, N - c0:]
        # mirror the unwritten blocks from their transposed twins
        for t in range(NT):
            rs = slice(t * 128, (t + 1) * 128)
            for db in range(NBLK, NT):
                s = (t + db) % NT
                cs = slice(s * 128, (s + 1) * 128)
                P[rs, cs] = P[cs, rs].T
        outs.append(P)
    return outs[0], outs[1]
